# revision 10
# baseline (speedup 1.0000x reference)
"""Trainium2 (8 NeuronCores) kernel for a 4-layer GCN + WeightAndSum readout + MLP head.

Strategy
--------
Nodes are sharded across the 8 cores, graph-aligned (128 graphs/core). Each
core owns the edges whose *destination* lands in its node range. On the host
(numpy, pure index preprocessing) the edges are sorted by destination and
packed into 128-edge "tiles"; each tile covers at most 16 destination nodes
("slots") and a node's edge run never crosses a tile boundary. 8 tiles form a
"group" of 128 slots.

On device, per GCN layer:
  pass 1 (per group):
    - indirect-DMA gather of the 1024 edge source rows (from h for layer 1,
      from the all-gathered feature buffer z for later layers)
    - 8 TensorE matmuls (lhsT = gathered messages [128e, F], rhs = per-tile
      indicator [128e, 16] holding ns[src]*nd[dst] at the edge's slot column)
      assemble the normalized aggregation transposed: zT [F, 128 slots]
    - one matmul yT = W.T @ zT  -> pre-BN features [H, slots] (b absorbed by BN)
    - BN statistics (sum / sum-of-squares over slots) accumulated per group
  tiny AllReduce of the BN stats, then BN scale/shift vectors are formed.
  pass 2 (per group): fused BN+ReLU (ScalarE, per-partition affine), TensorE
    transpose back to row layout, indirect-DMA scatter into the compact
    per-core feature shard, then an AllGather replicates shards into z.
Layer 4 additionally computes atom_weights = x@aw + ab (returned), gates rows
with sigmoid, and accumulates the per-graph readout hg with one more
indicator matmul per group into a persistent PSUM bank. The dense MLP head
(BN stats all-reduced, G=1024) runs data-parallel over graphs (128/core).
"""

import sys

import numpy as np

for _p in ("/opt/trn_rl_repo",):
    if _p not in sys.path:
        sys.path.insert(0, _p)

P = 128          # SBUF partitions / edges per tile
WIN = 16         # max distinct dst nodes (slots) per tile
TPG = 8          # tiles per group (8*16 = 128 slots)
GPB = 2          # groups per gather batch (16 tiles / indirect DMA)
SPB = 8          # groups per scatter batch
N_CORES = 8
EPS_BN = 1e-5


# ----------------------------------------------------------------------------
# Host preprocessing (numpy; index manipulation + degree-norm precompute)
# ----------------------------------------------------------------------------

def _preprocess(inputs, n_graphs):
    h = np.ascontiguousarray(np.asarray(inputs["h"], dtype=np.float32))
    src = np.asarray(inputs["src"]).astype(np.int64)
    dst = np.asarray(inputs["dst"]).astype(np.int64)
    n2g = np.asarray(inputs["node2graph"]).astype(np.int64)
    N, F = h.shape
    E = src.shape[0]
    G = n_graphs
    C = N_CORES
    GPC = G // C

    deg_out = np.bincount(src, minlength=N)
    deg_in = np.bincount(dst, minlength=N)
    ns = (1.0 / np.sqrt(np.maximum(deg_out, 1.0))).astype(np.float32)
    nd = (1.0 / np.sqrt(np.maximum(deg_in, 1.0))).astype(np.float32)

    # graph-aligned node ranges per core
    nb = np.searchsorted(n2g, np.arange(0, G + 1, GPC))  # [C+1]
    # edges sorted by dst; per-node contiguous runs
    order = np.argsort(dst, kind="stable")
    dst_s = dst[order]
    src_s = src[order]
    node_e_start = np.searchsorted(dst_s, np.arange(N + 1))  # run of node v: [s[v], s[v+1])

    # --- pack runs into tiles (one pass over all nodes; cores are contiguous)
    max_deg = int(deg_in.max(initial=0))
    assert max_deg <= P, f"node in-degree {max_deg} exceeds tile capacity"

    tile_of_node = np.empty(N, dtype=np.int64)   # per-core tile index
    slot_of_node = np.empty(N, dtype=np.int64)   # slot within tile [0, WIN)
    ebase_of_node = np.empty(N, dtype=np.int64)  # edge offset within tile
    tiles_per_core = np.zeros(C, dtype=np.int64)

    d_all = (node_e_start[1:] - node_e_start[:-1])
    for c in range(C):
        lo, hi = int(nb[c]), int(nb[c + 1])
        t = 0
        cur_e = 0
        cur_s = 0
        for v in range(lo, hi):
            d = int(d_all[v])
            if cur_e + d > P or cur_s + 1 > WIN:
                t += 1
                cur_e = 0
                cur_s = 0
            tile_of_node[v] = t
            slot_of_node[v] = cur_s
            ebase_of_node[v] = cur_e
            cur_e += d
            cur_s += 1
        tiles_per_core[c] = t + 1 if hi > lo else 0

    n_groups = int(np.max((tiles_per_core + TPG - 1) // TPG))
    lcm = (GPB * SPB) // np.gcd(GPB, SPB)
    G_MAX = int(-(-n_groups // lcm) * lcm)           # round up to lcm(2,8)=8
    NB = G_MAX // GPB
    SB = G_MAX // SPB
    M_PAD = int(np.max(nb[1:] - nb[:-1])) + 8        # z shard rows (+ trash)
    TRASH = M_PAD - 1
    n_slots = G_MAX * P

    gidx1 = np.zeros((C, NB, P, GPB * TPG), dtype=np.int32)
    gidx2 = np.zeros((C, NB, P, GPB * TPG), dtype=np.int32)
    ind = np.zeros((C, G_MAX, P, TPG * WIN), dtype=np.float32)
    indG = np.zeros((C, G_MAX, P, GPC), dtype=np.float32)
    scat = np.full((C, SB, P, SPB), TRASH, dtype=np.int32)
    slot2node = np.full((C, n_slots), -1, dtype=np.int64)

    core_of_node = np.searchsorted(nb, np.arange(N), side="right") - 1
    # per-edge placement (vectorized over sorted edges)
    e_node = dst_s                                    # dst node of each sorted edge
    e_core = core_of_node[e_node]
    e_tile = tile_of_node[e_node]
    e_rank = np.arange(E) - node_e_start[e_node]      # position within run
    e_part = ebase_of_node[e_node] + e_rank           # partition (edge row in tile)
    e_slot = slot_of_node[e_node]                     # slot col within tile
    e_w = ns[src_s] * nd[dst_s]

    e_group = e_tile // TPG
    e_jt = e_tile % TPG                               # tile within group
    e_batch = e_tile // (GPB * TPG)
    e_jb = e_tile % (GPB * TPG)                       # tile within gather batch

    ind[e_core, e_group, e_part, e_jt * WIN + e_slot] = e_w
    gidx1[e_core, e_batch, e_part, e_jb] = src_s.astype(np.int32)
    src_row2 = (core_of_node[src_s] * M_PAD + (src_s - nb[core_of_node[src_s]])).astype(np.int32)
    gidx2[e_core, e_batch, e_part, e_jb] = src_row2

    # per-node placement (slots; includes degree-0 nodes)
    v_all = np.arange(N)
    v_core = core_of_node
    v_group = tile_of_node // TPG
    v_q = (tile_of_node % TPG) * WIN + slot_of_node   # slot-in-group = partition after transpose
    v_local = v_all - nb[v_core]
    scat[v_core, v_group // SPB, v_q, v_group % SPB] = v_local.astype(np.int32)
    slot2node[v_core, v_group * P + v_q] = v_all
    indG[v_core, v_group, v_q, n2g - v_core * GPC] = 1.0

    return dict(
        N=N, F=F, E=E, G=G, C=C, GPC=GPC, G_MAX=G_MAX, NB=NB, SB=SB,
        M_PAD=M_PAD, n_slots=n_slots, nb=nb, h=h,
        gidx1=gidx1, gidx2=gidx2, ind=ind, indG=indG, scat=scat,
        slot2node=slot2node,
    )


# ----------------------------------------------------------------------------
# Bass/Tile program
# ----------------------------------------------------------------------------

def _build(cfg, depth):
    import concourse.bacc as bacc
    import concourse.tile as tile
    from concourse import bass, mybir
    from concourse.bass import IndirectOffsetOnAxis
    from concourse.masks import make_identity

    f32 = mybir.dt.float32
    i32 = mybir.dt.int32
    AF = mybir.ActivationFunctionType
    OP = mybir.AluOpType
    AX = mybir.AxisListType

    N, F, G = cfg["N"], cfg["F"], cfg["G"]
    H = 128
    GPC = cfg["GPC"]
    G_MAX, NB, SB, M_PAD = cfg["G_MAX"], cfg["NB"], cfg["SB"], cfg["M_PAD"]
    rg = [list(range(N_CORES))]

    nc = bacc.Bacc("TRN2", target_bir_lowering=False, debug=False,
                   enable_asserts=False, num_devices=N_CORES)

    # ---- I/O ----
    h_t = nc.dram_tensor("h", [N, F], f32, kind="ExternalInput")
    gidx1_t = nc.dram_tensor("gidx1", [NB, P, GPB * TPG], i32, kind="ExternalInput")
    gidx2_t = nc.dram_tensor("gidx2", [NB, P, GPB * TPG], i32, kind="ExternalInput")
    ind_t = nc.dram_tensor("ind", [G_MAX, P, TPG * WIN], f32, kind="ExternalInput")
    indG_t = nc.dram_tensor("indG", [G_MAX, P, GPC], f32, kind="ExternalInput")
    scat_t = nc.dram_tensor("scat", [SB, P, SPB], i32, kind="ExternalInput")

    W1_t = nc.dram_tensor("W1", [F, H], f32, kind="ExternalInput")
    g1_t = nc.dram_tensor("g1", [H], f32, kind="ExternalInput")
    be1_t = nc.dram_tensor("be1", [H], f32, kind="ExternalInput")
    W2s_t = nc.dram_tensor("W2s", [depth - 1, H, H], f32, kind="ExternalInput")
    g2s_t = nc.dram_tensor("g2s", [depth - 1, H], f32, kind="ExternalInput")
    be2s_t = nc.dram_tensor("be2s", [depth - 1, H], f32, kind="ExternalInput")
    aw_t = nc.dram_tensor("aw", [H, 1], f32, kind="ExternalInput")
    ab_t = nc.dram_tensor("ab", [1], f32, kind="ExternalInput")
    Wf1_t = nc.dram_tensor("Wf1", [H, 512], f32, kind="ExternalInput")
    gf1_t = nc.dram_tensor("gf1", [512], f32, kind="ExternalInput")
    bef1_t = nc.dram_tensor("bef1", [512], f32, kind="ExternalInput")
    Wl_t = nc.dram_tensor("Wl", [512, 256], f32, kind="ExternalInput")
    gl_t = nc.dram_tensor("gl", [256], f32, kind="ExternalInput")
    bel_t = nc.dram_tensor("bel", [256], f32, kind="ExternalInput")
    Wf2_t = nc.dram_tensor("Wf2", [256, 67], f32, kind="ExternalInput")
    bf2_t = nc.dram_tensor("bf2", [67], f32, kind="ExternalInput")

    y_out_t = nc.dram_tensor("y_out", [GPC, 67], f32, kind="ExternalOutput")
    atom_out_t = nc.dram_tensor("atom_out", [G_MAX * P, 1], f32, kind="ExternalOutput")

    with tile.TileContext(nc) as tc:
        with (
            tc.tile_pool(name="const", bufs=1) as const,
            tc.tile_pool(name="io", bufs=4) as io,
            tc.tile_pool(name="msgp", bufs=3) as msgp,
            tc.tile_pool(name="indp", bufs=3) as indp,
            tc.tile_pool(name="work", bufs=3) as work,
            tc.tile_pool(name="small", bufs=4) as small,
            tc.tile_pool(name="store", bufs=1) as store,
            tc.tile_pool(name="psA", bufs=2, space="PSUM") as psA,
            tc.tile_pool(name="psY", bufs=2, space="PSUM") as psY,
            tc.tile_pool(name="psX", bufs=2, space="PSUM") as psX,
            tc.tile_pool(name="psHG", bufs=1, space="PSUM") as psHG,
            tc.tile_pool(name="dram", bufs=1, space="DRAM") as dram,
        ):
            # ---- constants / weights in SBUF ----
            ident = const.tile([P, P], f32)
            make_identity(nc, ident[:])

            w_sb = []      # per-layer lhsT [F_l, H]
            gamma_sb = []
            beta_sb = []
            w0 = const.tile([F, H], f32, name="w0")
            nc.sync.dma_start(out=w0[:], in_=W1_t[:])
            w_sb.append(w0)
            gm = const.tile([P, 1], f32, name="gm0")
            nc.sync.dma_start(out=gm[:], in_=g1_t[:, None])
            gamma_sb.append(gm)
            bt = const.tile([P, 1], f32, name="bt0")
            nc.sync.dma_start(out=bt[:], in_=be1_t[:, None])
            beta_sb.append(bt)
            for i in range(depth - 1):
                wl_ = const.tile([H, H], f32, name=f"w{i + 1}")
                nc.sync.dma_start(out=wl_[:], in_=W2s_t[i])
                w_sb.append(wl_)
                gm = const.tile([P, 1], f32, name=f"gm{i + 1}")
                nc.sync.dma_start(out=gm[:], in_=g2s_t[i, :, None])
                gamma_sb.append(gm)
                bt = const.tile([P, 1], f32, name=f"bt{i + 1}")
                nc.sync.dma_start(out=bt[:], in_=be2s_t[i, :, None])
                beta_sb.append(bt)

            aw_sb = const.tile([H, 1], f32)
            nc.sync.dma_start(out=aw_sb[:], in_=aw_t[:])
            ab1_sb = const.tile([1, 1], f32)
            nc.sync.dma_start(out=ab1_sb[:], in_=ab_t[:, None])
            ones_row = const.tile([1, P], f32)
            nc.vector.memset(ones_row[:], 1.0)
            eps_sb = const.tile([P, 1], f32)
            nc.vector.memset(eps_sb[:], EPS_BN)
            # broadcast ab over 128 partitions via K=1 matmul
            ab_ps = psY.tile([P, 1], f32, tag="ymat")
            nc.tensor.matmul(ab_ps[:], lhsT=ones_row[:], rhs=ab1_sb[:], start=True, stop=True)
            ab_sb = const.tile([P, 1], f32)
            nc.vector.tensor_copy(out=ab_sb[:], in_=ab_ps[:])

            wf1_sb = const.tile([H, 512], f32)
            nc.sync.dma_start(out=wf1_sb[:], in_=Wf1_t[:])
            wl_sb = const.tile([P, 4 * 256], f32)
            nc.sync.dma_start(out=wl_sb[:].rearrange("p (a m) -> p a m", m=256),
                              in_=Wl_t.ap().rearrange("(a p) m -> p a m", p=P))
            wf2_sb = const.tile([P, 2 * 67], f32)
            nc.sync.dma_start(out=wf2_sb[:].rearrange("p (a m) -> p a m", m=67),
                              in_=Wf2_t.ap().rearrange("(a p) m -> p a m", p=P))
            gf1_sb = const.tile([P, 4], f32)
            nc.sync.dma_start(out=gf1_sb[:], in_=gf1_t.ap().rearrange("(a p) -> p a", p=P))
            bef1_sb = const.tile([P, 4], f32)
            nc.sync.dma_start(out=bef1_sb[:], in_=bef1_t.ap().rearrange("(a p) -> p a", p=P))
            gl_sb = const.tile([P, 2], f32)
            nc.sync.dma_start(out=gl_sb[:], in_=gl_t.ap().rearrange("(a p) -> p a", p=P))
            bel_sb = const.tile([P, 2], f32)
            nc.sync.dma_start(out=bel_sb[:], in_=bel_t.ap().rearrange("(a p) -> p a", p=P))
            bf2_sb = const.tile([67, 1], f32)
            nc.sync.dma_start(out=bf2_sb[:], in_=bf2_t[:, None])

            # ---- persistent buffers ----
            yt_store = store.tile([P, G_MAX * P], f32)       # pre-BN features (transposed)
            sum_cols = store.tile([P, G_MAX], f32, name="sum_cols")
            sq_cols = store.tile([P, G_MAX], f32, name="sq_cols")

            z_local = dram.tile([M_PAD, H], f32)
            z_fulls = [
                dram.tile([N_CORES * M_PAD, H], f32, name=f"z_full{i}",
                          addr_space="Shared")
                for i in range(depth - 1)
            ]
            har_in = dram.tile([P, 8], f32, name="har_in")
            har_out = dram.tile([P, 8], f32, name="har_out", addr_space="Shared")
            har2_in = dram.tile([P, 4], f32, name="har2_in")
            har2_out = dram.tile([P, 4], f32, name="har2_out", addr_space="Shared")

            hg_ps = psHG.tile([P, H], f32)

            # ============================ GCN layers ============================
            for l in range(depth):
                F_l = F if l == 0 else H
                gsrc = h_t.ap() if l == 0 else z_fulls[l - 1][:]
                gidx = gidx1_t if l == 0 else gidx2_t

                # ---------- pass 1: aggregate + transform + stats ----------
                for b in range(NB):
                    offs = io.tile([P, GPB * TPG], i32, tag="offs")
                    nc.sync.dma_start(out=offs[:], in_=gidx[b])
                    msg = msgp.tile([P, GPB * TPG * F_l], f32, tag="msg")
                    for jj_ in range(GPB * TPG):
                        nc.gpsimd.indirect_dma_start(
                            out=msg[:, jj_ * F_l:(jj_ + 1) * F_l], out_offset=None,
                            in_=gsrc,
                            in_offset=IndirectOffsetOnAxis(ap=offs[:, jj_:jj_ + 1], axis=0),
                        )
                    for gi in range(GPB):
                        g = GPB * b + gi
                        ind_sb = indp.tile([P, TPG * WIN], f32, tag="ind")
                        nc.sync.dma_start(out=ind_sb[:], in_=ind_t[g])
                        agg = psA.tile([P, P], f32, tag="agg")
                        for j in range(TPG):
                            jj = gi * TPG + j
                            nc.tensor.matmul(
                                agg[0:F_l, j * WIN:(j + 1) * WIN],
                                lhsT=msg[:, jj * F_l:(jj + 1) * F_l],
                                rhs=ind_sb[:, j * WIN:(j + 1) * WIN],
                                start=True, stop=True,
                            )
                        zT = work.tile([P, P], f32, tag="zT")
                        nc.scalar.activation(zT[0:F_l, :], agg[0:F_l, :], AF.Copy)
                        yT = psY.tile([P, P], f32, tag="ymat")
                        nc.tensor.matmul(yT[:], lhsT=w_sb[l][0:F_l, :], rhs=zT[0:F_l, :],
                                         start=True, stop=True)
                        nc.scalar.activation(
                            yt_store[:, g * P:(g + 1) * P], yT[:], AF.Copy,
                            accum_out=sum_cols[:, g:g + 1],
                        )
                        sqs = work.tile([P, P], f32, tag="sqs")
                        nc.vector.tensor_mul(sqs[:], yt_store[:, g * P:(g + 1) * P],
                                             yt_store[:, g * P:(g + 1) * P])
                        nc.vector.tensor_reduce(sq_cols[:, g:g + 1], sqs[:],
                                                axis=AX.X, op=OP.add)

                # ---------- BN stats all-reduce ----------
                ar_in = dram.tile([P, 2], f32, name=f"ar_in{l}")
                ar_out = dram.tile([P, 2], f32, name=f"ar_out{l}", addr_space="Shared")
                stat_sb = small.tile([P, 2], f32, tag="stat")
                nc.vector.tensor_reduce(stat_sb[:, 0:1], sum_cols[:], axis=AX.X, op=OP.add)
                nc.vector.tensor_reduce(stat_sb[:, 1:2], sq_cols[:], axis=AX.X, op=OP.add)
                nc.sync.dma_start(out=ar_in[:], in_=stat_sb[:])
                nc.gpsimd.collective_compute(
                    "AllReduce", OP.add, replica_groups=rg,
                    ins=[ar_in[:]], outs=[ar_out[:]],
                )
                statg = small.tile([P, 2], f32, tag="statg")
                nc.sync.dma_start(out=statg[:], in_=ar_out[:])

                mvec = small.tile([P, 1], f32, tag="mvec")
                nc.vector.tensor_scalar_mul(mvec[:], statg[:, 0:1], 1.0 / N)
                var = small.tile([P, 1], f32, tag="var")
                nc.vector.tensor_scalar_mul(var[:], statg[:, 1:2], 1.0 / N)
                m2 = small.tile([P, 1], f32, tag="m2")
                nc.vector.tensor_mul(m2[:], mvec[:], mvec[:])
                nc.vector.tensor_sub(var[:], var[:], m2[:])
                sd = small.tile([P, 1], f32, tag="sd")
                nc.scalar.activation(sd[:], var[:], AF.Sqrt, bias=eps_sb[:])
                rs = small.tile([P, 1], f32, tag="rs")
                nc.vector.reciprocal(rs[:], sd[:])
                svec = small.tile([P, 1], f32, tag="svec")
                nc.vector.tensor_mul(svec[:], rs[:], gamma_sb[l][:])
                tvec = small.tile([P, 1], f32, tag="tvec")
                nc.vector.tensor_mul(tvec[:], mvec[:], svec[:])
                nc.vector.tensor_sub(tvec[:], beta_sb[l][:], tvec[:])

                # ---------- pass 2: BN+ReLU, transpose, scatter / readout ----------
                last = l == depth - 1
                for sbi in range(SB):
                    if not last:
                        stage = msgp.tile([P, SPB * H], f32, tag="stage")
                    for j in range(SPB):
                        g = SPB * sbi + j
                        xT = work.tile([P, P], f32, tag="xT")
                        nc.scalar.activation(
                            xT[:], yt_store[:, g * P:(g + 1) * P], AF.Relu,
                            bias=tvec[:], scale=svec[:],
                        )
                        xr = psX.tile([P, P + 4], f32, tag="xr")
                        nc.tensor.transpose(xr[:, 0:P], xT[:], ident[:])
                        if last:
                            nc.tensor.matmul(xr[:, P:P + 1], lhsT=xT[:], rhs=aw_sb[:],
                                             start=True, stop=True)
                            atom_sb = small.tile([P, 1], f32, tag="atom")
                            nc.scalar.activation(atom_sb[:], xr[:, P:P + 1], AF.Identity,
                                                 bias=ab_sb[:])
                            nc.sync.dma_start(out=atom_out_t[g * P:(g + 1) * P, :],
                                              in_=atom_sb[:])
                            wgt = small.tile([P, 1], f32, tag="wgt")
                            nc.scalar.activation(wgt[:], xr[:, P:P + 1], AF.Sigmoid,
                                                 bias=ab_sb[:])
                            gated = work.tile([P, P], f32, tag="gated")
                            nc.vector.tensor_scalar_mul(gated[:], xr[:, 0:P], wgt[:])
                            indG_sb = indp.tile([P, GPC], f32, tag="indG")
                            nc.sync.dma_start(out=indG_sb[:], in_=indG_t[g])
                            nc.tensor.matmul(hg_ps[0:GPC, :], lhsT=indG_sb[:], rhs=gated[:],
                                             start=(g == 0), stop=(g == G_MAX - 1))
                        else:
                            nc.vector.tensor_copy(out=stage[:, j * H:(j + 1) * H],
                                                  in_=xr[:, 0:P])
                    if not last:
                        soffs = io.tile([P, SPB], i32, tag="soffs")
                        nc.sync.dma_start(out=soffs[:], in_=scat_t[sbi])
                        for j_ in range(SPB):
                            nc.gpsimd.indirect_dma_start(
                                out=z_local[:],
                                out_offset=IndirectOffsetOnAxis(ap=soffs[:, j_:j_ + 1], axis=0),
                                in_=stage[:, j_ * H:(j_ + 1) * H], in_offset=None,
                            )
                if not last:
                    nc.gpsimd.collective_compute(
                        "AllGather", mybir.AluOpType.bypass, replica_groups=rg,
                        ins=[z_local[:]], outs=[z_fulls[l][:]],
                    )

            # ============================ MLP head ============================
            hg_sb = work.tile([P, H], f32, tag="zT")
            nc.vector.tensor_copy(out=hg_sb[:], in_=hg_ps[:])
            hgT_ps = psA.tile([P, P], f32, tag="agg")
            nc.tensor.transpose(hgT_ps[:, 0:GPC], hg_sb[0:GPC, :], ident[0:GPC, 0:GPC])
            hgT_sb = work.tile([P, GPC], f32, tag="hgT")
            nc.scalar.activation(hgT_sb[:], hgT_ps[:, 0:GPC], AF.Copy)

            # FC1 (H -> 512) + BN + relu, transposed layout, 4 chunks of 128 ch
            y1_sb = store.tile([P, 4 * GPC], f32, name="y1_sb")
            hsum = small.tile([P, 8], f32, tag="hsum")
            for m in range(4):
                y1_ps = psY.tile([P, GPC], f32, tag="ymat")
                nc.tensor.matmul(y1_ps[:], lhsT=wf1_sb[:, m * 128:(m + 1) * 128],
                                 rhs=hgT_sb[:], start=True, stop=True)
                nc.scalar.activation(y1_sb[:, m * GPC:(m + 1) * GPC], y1_ps[:], AF.Copy,
                                     accum_out=hsum[:, m:m + 1])
                sq1 = work.tile([P, GPC], f32, tag="sqs")
                nc.vector.tensor_mul(sq1[:], y1_sb[:, m * GPC:(m + 1) * GPC],
                                     y1_sb[:, m * GPC:(m + 1) * GPC])
                nc.vector.tensor_reduce(hsum[:, 4 + m:5 + m], sq1[:],
                                        axis=AX.X, op=OP.add)
            nc.sync.dma_start(out=har_in[:], in_=hsum[:])
            nc.gpsimd.collective_compute("AllReduce", OP.add, replica_groups=rg,
                                         ins=[har_in[:]], outs=[har_out[:]])
            hstatg = small.tile([P, 8], f32, tag="hstatg")
            nc.sync.dma_start(out=hstatg[:], in_=har_out[:])

            x1_sb = store.tile([P, 4 * GPC], f32, name="x1_sb")
            for m in range(4):
                mv = small.tile([P, 1], f32, tag="mvec")
                nc.vector.tensor_scalar_mul(mv[:], hstatg[:, m:m + 1], 1.0 / G)
                vv = small.tile([P, 1], f32, tag="var")
                nc.vector.tensor_scalar_mul(vv[:], hstatg[:, 4 + m:5 + m], 1.0 / G)
                m2 = small.tile([P, 1], f32, tag="m2")
                nc.vector.tensor_mul(m2[:], mv[:], mv[:])
                nc.vector.tensor_sub(vv[:], vv[:], m2[:])
                sd = small.tile([P, 1], f32, tag="sd")
                nc.scalar.activation(sd[:], vv[:], AF.Sqrt, bias=eps_sb[:])
                rs = small.tile([P, 1], f32, tag="rs")
                nc.vector.reciprocal(rs[:], sd[:])
                sv = small.tile([P, 1], f32, tag="svec")
                nc.vector.tensor_mul(sv[:], rs[:], gf1_sb[:, m:m + 1])
                tv = small.tile([P, 1], f32, tag="tvec")
                nc.vector.tensor_mul(tv[:], mv[:], sv[:])
                nc.vector.tensor_sub(tv[:], bef1_sb[:, m:m + 1], tv[:])
                nc.scalar.activation(x1_sb[:, m * GPC:(m + 1) * GPC],
                                     y1_sb[:, m * GPC:(m + 1) * GPC], AF.Relu,
                                     bias=tv[:], scale=sv[:])

            # FC2 (512 -> 256) + BN + relu
            y2_sb = store.tile([P, 2 * GPC], f32, name="y2_sb")
            h2sum = small.tile([P, 4], f32, tag="h2sum")
            for m in range(2):
                y2_ps = psY.tile([P, GPC], f32, tag="ymat")
                for k in range(4):
                    nc.tensor.matmul(
                        y2_ps[:], lhsT=wl_sb[:, k * 256 + m * 128:k * 256 + (m + 1) * 128],
                        rhs=x1_sb[:, k * GPC:(k + 1) * GPC],
                        start=(k == 0), stop=(k == 3),
                    )
                nc.scalar.activation(y2_sb[:, m * GPC:(m + 1) * GPC], y2_ps[:], AF.Copy,
                                     accum_out=h2sum[:, m:m + 1])
                sq2 = work.tile([P, GPC], f32, tag="sqs")
                nc.vector.tensor_mul(sq2[:], y2_sb[:, m * GPC:(m + 1) * GPC],
                                     y2_sb[:, m * GPC:(m + 1) * GPC])
                nc.vector.tensor_reduce(h2sum[:, 2 + m:3 + m], sq2[:],
                                        axis=AX.X, op=OP.add)
            nc.sync.dma_start(out=har2_in[:], in_=h2sum[:])
            nc.gpsimd.collective_compute("AllReduce", OP.add, replica_groups=rg,
                                         ins=[har2_in[:]], outs=[har2_out[:]])
            h2statg = small.tile([P, 4], f32, tag="h2statg")
            nc.sync.dma_start(out=h2statg[:], in_=har2_out[:])

            x2_sb = store.tile([P, 2 * GPC], f32, name="x2_sb")
            for m in range(2):
                mv = small.tile([P, 1], f32, tag="mvec")
                nc.vector.tensor_scalar_mul(mv[:], h2statg[:, m:m + 1], 1.0 / G)
                vv = small.tile([P, 1], f32, tag="var")
                nc.vector.tensor_scalar_mul(vv[:], h2statg[:, 2 + m:3 + m], 1.0 / G)
                m2 = small.tile([P, 1], f32, tag="m2")
                nc.vector.tensor_mul(m2[:], mv[:], mv[:])
                nc.vector.tensor_sub(vv[:], vv[:], m2[:])
                sd = small.tile([P, 1], f32, tag="sd")
                nc.scalar.activation(sd[:], vv[:], AF.Sqrt, bias=eps_sb[:])
                rs = small.tile([P, 1], f32, tag="rs")
                nc.vector.reciprocal(rs[:], sd[:])
                sv = small.tile([P, 1], f32, tag="svec")
                nc.vector.tensor_mul(sv[:], rs[:], gl_sb[:, m:m + 1])
                tv = small.tile([P, 1], f32, tag="tvec")
                nc.vector.tensor_mul(tv[:], mv[:], sv[:])
                nc.vector.tensor_sub(tv[:], bel_sb[:, m:m + 1], tv[:])
                nc.scalar.activation(x2_sb[:, m * GPC:(m + 1) * GPC],
                                     y2_sb[:, m * GPC:(m + 1) * GPC], AF.Relu,
                                     bias=tv[:], scale=sv[:])

            # FC3 (256 -> 67) + sigmoid
            y3_ps = psY.tile([P, GPC], f32, tag="ymat")
            for k in range(2):
                nc.tensor.matmul(y3_ps[0:67, :], lhsT=wf2_sb[:, k * 67:(k + 1) * 67],
                                 rhs=x2_sb[:, k * GPC:(k + 1) * GPC],
                                 start=(k == 0), stop=(k == 1))
            y3_sb = work.tile([67, GPC], f32, tag="y3")
            nc.scalar.activation(y3_sb[:], y3_ps[0:67, :], AF.Sigmoid, bias=bf2_sb[:])
            yout_ps = psX.tile([P, 68], f32, tag="xr")
            nc.tensor.transpose(yout_ps[0:GPC, 0:67], y3_sb[:], ident[0:67, 0:67])
            yfin = work.tile([GPC, 67], f32, tag="yfin")
            nc.vector.tensor_copy(out=yfin[:], in_=yout_ps[0:GPC, 0:67])
            nc.sync.dma_start(out=y_out_t[:], in_=yfin[:])

    nc.compile()
    return nc


# ----------------------------------------------------------------------------
# glue
# ----------------------------------------------------------------------------

def _in_maps(cfg, inputs, depth):
    C = cfg["C"]
    f = np.float32
    base = dict(
        h=cfg["h"],
        W1=np.asarray(inputs["W1"], f), g1=np.asarray(inputs["g1"], f),
        be1=np.asarray(inputs["be1"], f),
        W2s=np.asarray(inputs["W2s"], f), g2s=np.asarray(inputs["g2s"], f),
        be2s=np.asarray(inputs["be2s"], f),
        aw=np.asarray(inputs["aw"], f), ab=np.asarray(inputs["ab"], f),
        Wf1=np.asarray(inputs["Wf1"], f), gf1=np.asarray(inputs["gf1"], f),
        bef1=np.asarray(inputs["bef1"], f),
        Wl=np.asarray(inputs["Wl"], f), gl=np.asarray(inputs["gl"], f),
        bel=np.asarray(inputs["bel"], f),
        Wf2=np.asarray(inputs["Wf2"], f), bf2=np.asarray(inputs["bf2"], f),
    )
    maps = []
    for c in range(C):
        m = dict(base)
        m["gidx1"] = np.ascontiguousarray(cfg["gidx1"][c])
        m["gidx2"] = np.ascontiguousarray(cfg["gidx2"][c])
        m["ind"] = np.ascontiguousarray(cfg["ind"][c])
        m["indG"] = np.ascontiguousarray(cfg["indG"][c])
        m["scat"] = np.ascontiguousarray(cfg["scat"][c])
        maps.append(m)
    return maps


def _postprocess(cfg, results):
    C, GPC, N = cfg["C"], cfg["GPC"], cfg["N"]
    y = np.concatenate([results[c]["y_out"] for c in range(C)], axis=0)
    atom = np.zeros((N, 1), dtype=np.float32)
    for c in range(C):
        s2n = cfg["slot2node"][c]
        real = s2n >= 0
        atom[s2n[real], 0] = results[c]["atom_out"][real, 0]
    return y, atom


def run(inputs, n_graphs=1024, depth=4, trace=False):
    from concourse import bass_utils
    cfg = _preprocess(inputs, n_graphs)
    nc = _build(cfg, depth)
    maps = _in_maps(cfg, inputs, depth)
    res = bass_utils.run_bass_kernel_spmd(
        nc, maps, core_ids=list(range(N_CORES)), trace=trace,
    )
    out = _postprocess(cfg, res.results)
    return out, res


def kernel(**inputs):
    (y, atom), _ = run(inputs)
    return y, atom


# revision 13
# speedup vs baseline: 1.0318x; 1.0318x over previous
"""Trainium2 (8 NeuronCores) kernel for a 4-layer GCN + WeightAndSum readout + MLP head.

Strategy
--------
Nodes are sharded across the 8 cores, graph-aligned (128 graphs/core). Each
core owns the edges whose *destination* lands in its node range. On the host
(numpy, pure index preprocessing) the edges are sorted by destination and
packed into 128-edge "tiles"; each tile covers at most 16 destination nodes
("slots") and a node's edge run never crosses a tile boundary. 8 tiles form a
"group" of 128 slots.

On device, per GCN layer:
  pass 1 (per group):
    - indirect-DMA gather of the 1024 edge source rows (from h for layer 1,
      from the all-gathered feature buffer z for later layers)
    - 8 TensorE matmuls (lhsT = gathered messages [128e, F], rhs = per-tile
      indicator [128e, 16] holding ns[src]*nd[dst] at the edge's slot column)
      assemble the normalized aggregation transposed: zT [F, 128 slots]
    - one matmul yT = W.T @ zT  -> pre-BN features [H, slots] (b absorbed by BN)
    - BN statistics (sum / sum-of-squares over slots) accumulated per group
  tiny AllReduce of the BN stats, then BN scale/shift vectors are formed.
  pass 2 (per group): fused BN+ReLU (ScalarE, per-partition affine), TensorE
    transpose back to row layout, indirect-DMA scatter into the compact
    per-core feature shard, then an AllGather replicates shards into z.
Layer 4 additionally computes atom_weights = x@aw + ab (returned), gates rows
with sigmoid, and accumulates the per-graph readout hg with one more
indicator matmul per group into a persistent PSUM bank. The dense MLP head
(BN stats all-reduced, G=1024) runs data-parallel over graphs (128/core).
"""

import sys

import numpy as np

for _p in ("/opt/trn_rl_repo",):
    if _p not in sys.path:
        sys.path.insert(0, _p)

P = 128          # SBUF partitions / edges per tile
WIN = 16         # max distinct dst nodes (slots) per tile
TPG = 8          # tiles per group (8*16 = 128 slots)
GPB = 2          # groups per gather batch (16 tiles / indirect DMA)
SPB = 8          # groups per scatter batch
N_CORES = 8
EPS_BN = 1e-5


# ----------------------------------------------------------------------------
# Host preprocessing (numpy; index manipulation + degree-norm precompute)
# ----------------------------------------------------------------------------

def _preprocess(inputs, n_graphs):
    h = np.ascontiguousarray(np.asarray(inputs["h"], dtype=np.float32))
    src = np.asarray(inputs["src"]).astype(np.int64)
    dst = np.asarray(inputs["dst"]).astype(np.int64)
    n2g = np.asarray(inputs["node2graph"]).astype(np.int64)
    N, F = h.shape
    E = src.shape[0]
    G = n_graphs
    C = N_CORES
    GPC = G // C

    deg_out = np.bincount(src, minlength=N)
    deg_in = np.bincount(dst, minlength=N)
    ns = (1.0 / np.sqrt(np.maximum(deg_out, 1.0))).astype(np.float32)
    nd = (1.0 / np.sqrt(np.maximum(deg_in, 1.0))).astype(np.float32)

    # graph-aligned node ranges per core
    nb = np.searchsorted(n2g, np.arange(0, G + 1, GPC))  # [C+1]
    # edges sorted by dst; per-node contiguous runs
    order = np.argsort(dst, kind="stable")
    dst_s = dst[order]
    src_s = src[order]
    node_e_start = np.searchsorted(dst_s, np.arange(N + 1))  # run of node v: [s[v], s[v+1])

    # --- pack runs into tiles (one pass over all nodes; cores are contiguous)
    max_deg = int(deg_in.max(initial=0))
    assert max_deg <= P, f"node in-degree {max_deg} exceeds tile capacity"

    tile_of_node = np.empty(N, dtype=np.int64)   # per-core tile index
    slot_of_node = np.empty(N, dtype=np.int64)   # slot within tile [0, WIN)
    ebase_of_node = np.empty(N, dtype=np.int64)  # edge offset within tile
    tiles_per_core = np.zeros(C, dtype=np.int64)

    d_all = (node_e_start[1:] - node_e_start[:-1])
    for c in range(C):
        lo, hi = int(nb[c]), int(nb[c + 1])
        t = 0
        cur_e = 0
        cur_s = 0
        for v in range(lo, hi):
            d = int(d_all[v])
            if cur_e + d > P or cur_s + 1 > WIN:
                t += 1
                cur_e = 0
                cur_s = 0
            tile_of_node[v] = t
            slot_of_node[v] = cur_s
            ebase_of_node[v] = cur_e
            cur_e += d
            cur_s += 1
        tiles_per_core[c] = t + 1 if hi > lo else 0

    n_groups = int(np.max((tiles_per_core + TPG - 1) // TPG))
    lcm = (GPB * SPB) // np.gcd(GPB, SPB)
    G_MAX = int(-(-n_groups // lcm) * lcm)           # round up to lcm(2,8)=8
    NB = G_MAX // GPB
    SB = G_MAX // SPB
    M_PAD = int(np.max(nb[1:] - nb[:-1])) + 8        # z shard rows (+ trash)
    TRASH = M_PAD - 1
    n_slots = G_MAX * P

    gidx1 = np.zeros((C, NB, P, GPB * TPG), dtype=np.int32)
    gidx2 = np.zeros((C, NB, P, GPB * TPG), dtype=np.int32)
    ind = np.zeros((C, G_MAX, P, TPG * WIN), dtype=np.float32)
    indG = np.zeros((C, G_MAX, P, GPC), dtype=np.float32)
    scat = np.full((C, SB, P, SPB), TRASH, dtype=np.int32)
    slot2node = np.full((C, n_slots), -1, dtype=np.int64)

    core_of_node = np.searchsorted(nb, np.arange(N), side="right") - 1
    # per-edge placement (vectorized over sorted edges)
    e_node = dst_s                                    # dst node of each sorted edge
    e_core = core_of_node[e_node]
    e_tile = tile_of_node[e_node]
    e_rank = np.arange(E) - node_e_start[e_node]      # position within run
    e_part = ebase_of_node[e_node] + e_rank           # partition (edge row in tile)
    e_slot = slot_of_node[e_node]                     # slot col within tile
    e_w = ns[src_s] * nd[dst_s]

    e_group = e_tile // TPG
    e_jt = e_tile % TPG                               # tile within group
    e_batch = e_tile // (GPB * TPG)
    e_jb = e_tile % (GPB * TPG)                       # tile within gather batch

    ind[e_core, e_group, e_part, e_jt * WIN + e_slot] = e_w
    gidx1[e_core, e_batch, e_part, e_jb] = src_s.astype(np.int32)
    slot_of = (tile_of_node // TPG) * P + (tile_of_node % TPG) * WIN + slot_of_node
    src_row2 = (core_of_node[src_s] * n_slots + slot_of[src_s]).astype(np.int32)
    gidx2[e_core, e_batch, e_part, e_jb] = src_row2

    # per-node placement (slots; includes degree-0 nodes)
    v_all = np.arange(N)
    v_core = core_of_node
    v_group = tile_of_node // TPG
    v_q = (tile_of_node % TPG) * WIN + slot_of_node   # slot-in-group = partition after transpose
    v_local = v_all - nb[v_core]
    scat[v_core, v_group // SPB, v_q, v_group % SPB] = v_local.astype(np.int32)
    slot2node[v_core, v_group * P + v_q] = v_all
    indG[v_core, v_group, v_q, n2g - v_core * GPC] = 1.0

    return dict(
        N=N, F=F, E=E, G=G, C=C, GPC=GPC, G_MAX=G_MAX, NB=NB, SB=SB,
        M_PAD=M_PAD, n_slots=n_slots, nb=nb, h=h,
        gidx1=gidx1, gidx2=gidx2, ind=ind, indG=indG, scat=scat,
        slot2node=slot2node,
    )


# ----------------------------------------------------------------------------
# Bass/Tile program
# ----------------------------------------------------------------------------

def _build(cfg, depth):
    import concourse.bacc as bacc
    import concourse.tile as tile
    from concourse import bass, mybir
    from concourse.bass import IndirectOffsetOnAxis
    from concourse.masks import make_identity

    f32 = mybir.dt.float32
    i32 = mybir.dt.int32
    AF = mybir.ActivationFunctionType
    OP = mybir.AluOpType
    AX = mybir.AxisListType

    N, F, G = cfg["N"], cfg["F"], cfg["G"]
    H = 128
    GPC = cfg["GPC"]
    G_MAX, NB, SB, M_PAD = cfg["G_MAX"], cfg["NB"], cfg["SB"], cfg["M_PAD"]
    rg = [list(range(N_CORES))]

    nc = bacc.Bacc("TRN2", target_bir_lowering=False, debug=False,
                   enable_asserts=False, num_devices=N_CORES)

    # ---- I/O ----
    h_t = nc.dram_tensor("h", [N, F], f32, kind="ExternalInput")
    gidx1_t = nc.dram_tensor("gidx1", [NB, P, GPB * TPG], i32, kind="ExternalInput")
    gidx2_t = nc.dram_tensor("gidx2", [NB, P, GPB * TPG], i32, kind="ExternalInput")
    ind_t = nc.dram_tensor("ind", [G_MAX, P, TPG * WIN], f32, kind="ExternalInput")
    indG_t = nc.dram_tensor("indG", [G_MAX, P, GPC], f32, kind="ExternalInput")

    W1_t = nc.dram_tensor("W1", [F, H], f32, kind="ExternalInput")
    g1_t = nc.dram_tensor("g1", [H], f32, kind="ExternalInput")
    be1_t = nc.dram_tensor("be1", [H], f32, kind="ExternalInput")
    W2s_t = nc.dram_tensor("W2s", [depth - 1, H, H], f32, kind="ExternalInput")
    g2s_t = nc.dram_tensor("g2s", [depth - 1, H], f32, kind="ExternalInput")
    be2s_t = nc.dram_tensor("be2s", [depth - 1, H], f32, kind="ExternalInput")
    aw_t = nc.dram_tensor("aw", [H, 1], f32, kind="ExternalInput")
    ab_t = nc.dram_tensor("ab", [1], f32, kind="ExternalInput")
    Wf1_t = nc.dram_tensor("Wf1", [H, 512], f32, kind="ExternalInput")
    gf1_t = nc.dram_tensor("gf1", [512], f32, kind="ExternalInput")
    bef1_t = nc.dram_tensor("bef1", [512], f32, kind="ExternalInput")
    Wl_t = nc.dram_tensor("Wl", [512, 256], f32, kind="ExternalInput")
    gl_t = nc.dram_tensor("gl", [256], f32, kind="ExternalInput")
    bel_t = nc.dram_tensor("bel", [256], f32, kind="ExternalInput")
    Wf2_t = nc.dram_tensor("Wf2", [256, 67], f32, kind="ExternalInput")
    bf2_t = nc.dram_tensor("bf2", [67], f32, kind="ExternalInput")

    y_out_t = nc.dram_tensor("y_out", [GPC, 67], f32, kind="ExternalOutput")
    atom_out_t = nc.dram_tensor("atom_out", [G_MAX * P, 1], f32, kind="ExternalOutput")

    with tile.TileContext(nc) as tc:
        with (
            tc.tile_pool(name="const", bufs=1) as const,
            tc.tile_pool(name="io", bufs=8) as io,
            tc.tile_pool(name="msgp", bufs=3) as msgp,
            tc.tile_pool(name="indp", bufs=4) as indp,
            tc.tile_pool(name="work", bufs=3) as work,
            tc.tile_pool(name="small", bufs=4) as small,
            tc.tile_pool(name="store", bufs=1) as store,
            tc.tile_pool(name="psA", bufs=2, space="PSUM") as psA,
            tc.tile_pool(name="psY", bufs=2, space="PSUM") as psY,
            tc.tile_pool(name="psX", bufs=2, space="PSUM") as psX,
            tc.tile_pool(name="psHG", bufs=1, space="PSUM") as psHG,
            tc.tile_pool(name="dram", bufs=1, space="DRAM") as dram,
        ):
            # ---- constants / weights in SBUF ----
            ident = const.tile([P, P], f32)
            make_identity(nc, ident[:])

            w_sb = []      # per-layer lhsT [F_l, H]
            gamma_sb = []
            beta_sb = []
            w0 = const.tile([F, H], f32, name="w0")
            nc.sync.dma_start(out=w0[:], in_=W1_t[:])
            w_sb.append(w0)
            gm = const.tile([P, 1], f32, name="gm0")
            nc.sync.dma_start(out=gm[:], in_=g1_t[:, None])
            gamma_sb.append(gm)
            bt = const.tile([P, 1], f32, name="bt0")
            nc.sync.dma_start(out=bt[:], in_=be1_t[:, None])
            beta_sb.append(bt)
            for i in range(depth - 1):
                wl_ = const.tile([H, H], f32, name=f"w{i + 1}")
                nc.sync.dma_start(out=wl_[:], in_=W2s_t[i])
                w_sb.append(wl_)
                gm = const.tile([P, 1], f32, name=f"gm{i + 1}")
                nc.sync.dma_start(out=gm[:], in_=g2s_t[i, :, None])
                gamma_sb.append(gm)
                bt = const.tile([P, 1], f32, name=f"bt{i + 1}")
                nc.sync.dma_start(out=bt[:], in_=be2s_t[i, :, None])
                beta_sb.append(bt)

            aw_sb = const.tile([H, 1], f32)
            nc.sync.dma_start(out=aw_sb[:], in_=aw_t[:])
            ab1_sb = const.tile([1, 1], f32)
            nc.sync.dma_start(out=ab1_sb[:], in_=ab_t[:, None])
            ones_row = const.tile([1, P], f32)
            nc.vector.memset(ones_row[:], 1.0)
            eps_sb = const.tile([P, 1], f32)
            nc.vector.memset(eps_sb[:], EPS_BN)
            # broadcast ab over 128 partitions via K=1 matmul
            ab_ps = psY.tile([P, 1], f32, tag="ymat")
            nc.tensor.matmul(ab_ps[:], lhsT=ones_row[:], rhs=ab1_sb[:], start=True, stop=True)
            ab_sb = const.tile([P, 1], f32)
            nc.vector.tensor_copy(out=ab_sb[:], in_=ab_ps[:])

            wf1_sb = const.tile([H, 512], f32)
            nc.sync.dma_start(out=wf1_sb[:], in_=Wf1_t[:])
            wl_sb = const.tile([P, 4 * 256], f32)
            nc.sync.dma_start(out=wl_sb[:].rearrange("p (a m) -> p a m", m=256),
                              in_=Wl_t.ap().rearrange("(a p) m -> p a m", p=P))
            wf2_sb = const.tile([P, 2 * 67], f32)
            nc.sync.dma_start(out=wf2_sb[:].rearrange("p (a m) -> p a m", m=67),
                              in_=Wf2_t.ap().rearrange("(a p) m -> p a m", p=P))
            gf1_sb = const.tile([P, 4], f32)
            nc.sync.dma_start(out=gf1_sb[:], in_=gf1_t.ap().rearrange("(a p) -> p a", p=P))
            bef1_sb = const.tile([P, 4], f32)
            nc.sync.dma_start(out=bef1_sb[:], in_=bef1_t.ap().rearrange("(a p) -> p a", p=P))
            gl_sb = const.tile([P, 2], f32)
            nc.sync.dma_start(out=gl_sb[:], in_=gl_t.ap().rearrange("(a p) -> p a", p=P))
            bel_sb = const.tile([P, 2], f32)
            nc.sync.dma_start(out=bel_sb[:], in_=bel_t.ap().rearrange("(a p) -> p a", p=P))
            bf2_sb = const.tile([67, 1], f32)
            nc.sync.dma_start(out=bf2_sb[:], in_=bf2_t[:, None])

            # ---- persistent buffers ----
            yt_store = store.tile([P, G_MAX * P], f32)       # pre-BN features (transposed)
            sum_cols = store.tile([P, G_MAX], f32, name="sum_cols")
            sq_cols = store.tile([P, G_MAX], f32, name="sq_cols")

            NS = G_MAX * P
            z_local = dram.tile([NS, H], f32)
            z_fulls = [
                dram.tile([N_CORES * NS, H], f32, name=f"z_full{i}",
                          addr_space="Shared")
                for i in range(depth - 1)
            ]
            har_in = dram.tile([P, 8], f32, name="har_in")
            har_out = dram.tile([P, 8], f32, name="har_out", addr_space="Shared")
            har2_in = dram.tile([P, 4], f32, name="har2_in")
            har2_out = dram.tile([P, 4], f32, name="har2_out", addr_space="Shared")

            hg_ps = psHG.tile([P, H], f32)

            # ============================ GCN layers ============================
            for l in range(depth):
                F_l = F if l == 0 else H
                gsrc = h_t.ap() if l == 0 else z_fulls[l - 1][:]
                gidx = gidx1_t if l == 0 else gidx2_t

                # ---------- pass 1: aggregate + transform + stats ----------
                for b in range(NB):
                    offs = io.tile([P, GPB * TPG], i32, tag="offs")
                    nc.sync.dma_start(out=offs[:], in_=gidx[b])
                    msg = msgp.tile([P, GPB * TPG * F_l], f32, tag="msg")
                    for jj_ in range(GPB * TPG):
                        nc.gpsimd.indirect_dma_start(
                            out=msg[:, jj_ * F_l:(jj_ + 1) * F_l], out_offset=None,
                            in_=gsrc,
                            in_offset=IndirectOffsetOnAxis(ap=offs[:, jj_:jj_ + 1], axis=0),
                        )
                    for gi in range(GPB):
                        g = GPB * b + gi
                        ind_sb = indp.tile([P, TPG * WIN], f32, tag="ind")
                        nc.sync.dma_start(out=ind_sb[:], in_=ind_t[g])
                        agg = psA.tile([P, P], f32, tag="agg")
                        for j in range(TPG):
                            jj = gi * TPG + j
                            nc.tensor.matmul(
                                agg[0:F_l, j * WIN:(j + 1) * WIN],
                                lhsT=msg[:, jj * F_l:(jj + 1) * F_l],
                                rhs=ind_sb[:, j * WIN:(j + 1) * WIN],
                                start=True, stop=True,
                            )
                        zT = work.tile([P, P], f32, tag="zT")
                        nc.scalar.activation(zT[0:F_l, :], agg[0:F_l, :], AF.Copy)
                        yT = psY.tile([P, P], f32, tag="ymat")
                        nc.tensor.matmul(yT[:], lhsT=w_sb[l][0:F_l, :], rhs=zT[0:F_l, :],
                                         start=True, stop=True)
                        nc.scalar.activation(
                            yt_store[:, g * P:(g + 1) * P], yT[:], AF.Copy,
                            accum_out=sum_cols[:, g:g + 1],
                        )
                        sqs = work.tile([P, P], f32, tag="sqs")
                        nc.vector.tensor_mul(sqs[:], yt_store[:, g * P:(g + 1) * P],
                                             yt_store[:, g * P:(g + 1) * P])
                        nc.vector.tensor_reduce(sq_cols[:, g:g + 1], sqs[:],
                                                axis=AX.X, op=OP.add)

                # ---------- BN stats all-reduce ----------
                ar_in = dram.tile([P, 2], f32, name=f"ar_in{l}")
                ar_out = dram.tile([P, 2], f32, name=f"ar_out{l}", addr_space="Shared")
                stat_sb = small.tile([P, 2], f32, tag="stat")
                nc.vector.tensor_reduce(stat_sb[:, 0:1], sum_cols[:], axis=AX.X, op=OP.add)
                nc.vector.tensor_reduce(stat_sb[:, 1:2], sq_cols[:], axis=AX.X, op=OP.add)
                nc.sync.dma_start(out=ar_in[:], in_=stat_sb[:])
                nc.gpsimd.collective_compute(
                    "AllReduce", OP.add, replica_groups=rg,
                    ins=[ar_in[:]], outs=[ar_out[:]],
                )
                statg = small.tile([P, 2], f32, tag="statg")
                nc.sync.dma_start(out=statg[:], in_=ar_out[:])

                mvec = small.tile([P, 1], f32, tag="mvec")
                nc.vector.tensor_scalar_mul(mvec[:], statg[:, 0:1], 1.0 / N)
                var = small.tile([P, 1], f32, tag="var")
                nc.vector.tensor_scalar_mul(var[:], statg[:, 1:2], 1.0 / N)
                m2 = small.tile([P, 1], f32, tag="m2")
                nc.vector.tensor_mul(m2[:], mvec[:], mvec[:])
                nc.vector.tensor_sub(var[:], var[:], m2[:])
                sd = small.tile([P, 1], f32, tag="sd")
                nc.scalar.activation(sd[:], var[:], AF.Sqrt, bias=eps_sb[:])
                rs = small.tile([P, 1], f32, tag="rs")
                nc.vector.reciprocal(rs[:], sd[:])
                svec = small.tile([P, 1], f32, tag="svec")
                nc.vector.tensor_mul(svec[:], rs[:], gamma_sb[l][:])
                tvec = small.tile([P, 1], f32, tag="tvec")
                nc.vector.tensor_mul(tvec[:], mvec[:], svec[:])
                nc.vector.tensor_sub(tvec[:], beta_sb[l][:], tvec[:])

                # ---------- pass 2: BN+ReLU, transpose, scatter / readout ----------
                last = l == depth - 1
                for sbi in range(SB):
                    if not last:
                        stage = msgp.tile([P, SPB * H], f32, tag="stage")
                    for j in range(SPB):
                        g = SPB * sbi + j
                        xT = work.tile([P, P], f32, tag="xT")
                        nc.scalar.activation(
                            xT[:], yt_store[:, g * P:(g + 1) * P], AF.Relu,
                            bias=tvec[:], scale=svec[:],
                        )
                        xr = psX.tile([P, P + 4], f32, tag="xr")
                        nc.tensor.transpose(xr[:, 0:P], xT[:], ident[:])
                        if last:
                            nc.tensor.matmul(xr[:, P:P + 1], lhsT=xT[:], rhs=aw_sb[:],
                                             start=True, stop=True)
                            atom_sb = small.tile([P, 1], f32, tag="atom")
                            nc.scalar.activation(atom_sb[:], xr[:, P:P + 1], AF.Identity,
                                                 bias=ab_sb[:])
                            nc.sync.dma_start(out=atom_out_t[g * P:(g + 1) * P, :],
                                              in_=atom_sb[:])
                            wgt = small.tile([P, 1], f32, tag="wgt")
                            nc.scalar.activation(wgt[:], xr[:, P:P + 1], AF.Sigmoid,
                                                 bias=ab_sb[:])
                            gated = work.tile([P, P], f32, tag="gated")
                            nc.vector.tensor_scalar_mul(gated[:], xr[:, 0:P], wgt[:])
                            indG_sb = indp.tile([P, GPC], f32, tag="indG")
                            nc.sync.dma_start(out=indG_sb[:], in_=indG_t[g])
                            nc.tensor.matmul(hg_ps[0:GPC, :], lhsT=indG_sb[:], rhs=gated[:],
                                             start=(g == 0), stop=(g == G_MAX - 1))
                        else:
                            nc.vector.tensor_copy(out=stage[:, j * H:(j + 1) * H],
                                                  in_=xr[:, 0:P])
                    if not last:
                        nc.sync.dma_start(
                            out=z_local[sbi * SPB * P:(sbi + 1) * SPB * P, :]
                            .rearrange("(j p) h -> p j h", p=P),
                            in_=stage[:].rearrange("p (j h) -> p j h", h=H),
                        )
                if not last:
                    nc.gpsimd.collective_compute(
                        "AllGather", mybir.AluOpType.bypass, replica_groups=rg,
                        ins=[z_local[:]], outs=[z_fulls[l][:]],
                    )

            # ============================ MLP head ============================
            hg_sb = work.tile([P, H], f32, tag="zT")
            nc.vector.tensor_copy(out=hg_sb[:], in_=hg_ps[:])
            hgT_ps = psA.tile([P, P], f32, tag="agg")
            nc.tensor.transpose(hgT_ps[:, 0:GPC], hg_sb[0:GPC, :], ident[0:GPC, 0:GPC])
            hgT_sb = work.tile([P, GPC], f32, tag="hgT")
            nc.scalar.activation(hgT_sb[:], hgT_ps[:, 0:GPC], AF.Copy)

            # FC1 (H -> 512) + BN + relu, transposed layout, 4 chunks of 128 ch
            y1_sb = store.tile([P, 4 * GPC], f32, name="y1_sb")
            hsum = small.tile([P, 8], f32, tag="hsum")
            for m in range(4):
                y1_ps = psY.tile([P, GPC], f32, tag="ymat")
                nc.tensor.matmul(y1_ps[:], lhsT=wf1_sb[:, m * 128:(m + 1) * 128],
                                 rhs=hgT_sb[:], start=True, stop=True)
                nc.scalar.activation(y1_sb[:, m * GPC:(m + 1) * GPC], y1_ps[:], AF.Copy,
                                     accum_out=hsum[:, m:m + 1])
                sq1 = work.tile([P, GPC], f32, tag="sqs")
                nc.vector.tensor_mul(sq1[:], y1_sb[:, m * GPC:(m + 1) * GPC],
                                     y1_sb[:, m * GPC:(m + 1) * GPC])
                nc.vector.tensor_reduce(hsum[:, 4 + m:5 + m], sq1[:],
                                        axis=AX.X, op=OP.add)
            nc.sync.dma_start(out=har_in[:], in_=hsum[:])
            nc.gpsimd.collective_compute("AllReduce", OP.add, replica_groups=rg,
                                         ins=[har_in[:]], outs=[har_out[:]])
            hstatg = small.tile([P, 8], f32, tag="hstatg")
            nc.sync.dma_start(out=hstatg[:], in_=har_out[:])

            x1_sb = store.tile([P, 4 * GPC], f32, name="x1_sb")
            for m in range(4):
                mv = small.tile([P, 1], f32, tag="mvec")
                nc.vector.tensor_scalar_mul(mv[:], hstatg[:, m:m + 1], 1.0 / G)
                vv = small.tile([P, 1], f32, tag="var")
                nc.vector.tensor_scalar_mul(vv[:], hstatg[:, 4 + m:5 + m], 1.0 / G)
                m2 = small.tile([P, 1], f32, tag="m2")
                nc.vector.tensor_mul(m2[:], mv[:], mv[:])
                nc.vector.tensor_sub(vv[:], vv[:], m2[:])
                sd = small.tile([P, 1], f32, tag="sd")
                nc.scalar.activation(sd[:], vv[:], AF.Sqrt, bias=eps_sb[:])
                rs = small.tile([P, 1], f32, tag="rs")
                nc.vector.reciprocal(rs[:], sd[:])
                sv = small.tile([P, 1], f32, tag="svec")
                nc.vector.tensor_mul(sv[:], rs[:], gf1_sb[:, m:m + 1])
                tv = small.tile([P, 1], f32, tag="tvec")
                nc.vector.tensor_mul(tv[:], mv[:], sv[:])
                nc.vector.tensor_sub(tv[:], bef1_sb[:, m:m + 1], tv[:])
                nc.scalar.activation(x1_sb[:, m * GPC:(m + 1) * GPC],
                                     y1_sb[:, m * GPC:(m + 1) * GPC], AF.Relu,
                                     bias=tv[:], scale=sv[:])

            # FC2 (512 -> 256) + BN + relu
            y2_sb = store.tile([P, 2 * GPC], f32, name="y2_sb")
            h2sum = small.tile([P, 4], f32, tag="h2sum")
            for m in range(2):
                y2_ps = psY.tile([P, GPC], f32, tag="ymat")
                for k in range(4):
                    nc.tensor.matmul(
                        y2_ps[:], lhsT=wl_sb[:, k * 256 + m * 128:k * 256 + (m + 1) * 128],
                        rhs=x1_sb[:, k * GPC:(k + 1) * GPC],
                        start=(k == 0), stop=(k == 3),
                    )
                nc.scalar.activation(y2_sb[:, m * GPC:(m + 1) * GPC], y2_ps[:], AF.Copy,
                                     accum_out=h2sum[:, m:m + 1])
                sq2 = work.tile([P, GPC], f32, tag="sqs")
                nc.vector.tensor_mul(sq2[:], y2_sb[:, m * GPC:(m + 1) * GPC],
                                     y2_sb[:, m * GPC:(m + 1) * GPC])
                nc.vector.tensor_reduce(h2sum[:, 2 + m:3 + m], sq2[:],
                                        axis=AX.X, op=OP.add)
            nc.sync.dma_start(out=har2_in[:], in_=h2sum[:])
            nc.gpsimd.collective_compute("AllReduce", OP.add, replica_groups=rg,
                                         ins=[har2_in[:]], outs=[har2_out[:]])
            h2statg = small.tile([P, 4], f32, tag="h2statg")
            nc.sync.dma_start(out=h2statg[:], in_=har2_out[:])

            x2_sb = store.tile([P, 2 * GPC], f32, name="x2_sb")
            for m in range(2):
                mv = small.tile([P, 1], f32, tag="mvec")
                nc.vector.tensor_scalar_mul(mv[:], h2statg[:, m:m + 1], 1.0 / G)
                vv = small.tile([P, 1], f32, tag="var")
                nc.vector.tensor_scalar_mul(vv[:], h2statg[:, 2 + m:3 + m], 1.0 / G)
                m2 = small.tile([P, 1], f32, tag="m2")
                nc.vector.tensor_mul(m2[:], mv[:], mv[:])
                nc.vector.tensor_sub(vv[:], vv[:], m2[:])
                sd = small.tile([P, 1], f32, tag="sd")
                nc.scalar.activation(sd[:], vv[:], AF.Sqrt, bias=eps_sb[:])
                rs = small.tile([P, 1], f32, tag="rs")
                nc.vector.reciprocal(rs[:], sd[:])
                sv = small.tile([P, 1], f32, tag="svec")
                nc.vector.tensor_mul(sv[:], rs[:], gl_sb[:, m:m + 1])
                tv = small.tile([P, 1], f32, tag="tvec")
                nc.vector.tensor_mul(tv[:], mv[:], sv[:])
                nc.vector.tensor_sub(tv[:], bel_sb[:, m:m + 1], tv[:])
                nc.scalar.activation(x2_sb[:, m * GPC:(m + 1) * GPC],
                                     y2_sb[:, m * GPC:(m + 1) * GPC], AF.Relu,
                                     bias=tv[:], scale=sv[:])

            # FC3 (256 -> 67) + sigmoid
            y3_ps = psY.tile([P, GPC], f32, tag="ymat")
            for k in range(2):
                nc.tensor.matmul(y3_ps[0:67, :], lhsT=wf2_sb[:, k * 67:(k + 1) * 67],
                                 rhs=x2_sb[:, k * GPC:(k + 1) * GPC],
                                 start=(k == 0), stop=(k == 1))
            y3_sb = work.tile([67, GPC], f32, tag="y3")
            nc.scalar.activation(y3_sb[:], y3_ps[0:67, :], AF.Sigmoid, bias=bf2_sb[:])
            yout_ps = psX.tile([P, 68], f32, tag="xr")
            nc.tensor.transpose(yout_ps[0:GPC, 0:67], y3_sb[:], ident[0:67, 0:67])
            yfin = work.tile([GPC, 67], f32, tag="yfin")
            nc.vector.tensor_copy(out=yfin[:], in_=yout_ps[0:GPC, 0:67])
            nc.sync.dma_start(out=y_out_t[:], in_=yfin[:])

    nc.compile()
    return nc


# ----------------------------------------------------------------------------
# glue
# ----------------------------------------------------------------------------

def _in_maps(cfg, inputs, depth):
    C = cfg["C"]
    f = np.float32
    base = dict(
        h=cfg["h"],
        W1=np.asarray(inputs["W1"], f), g1=np.asarray(inputs["g1"], f),
        be1=np.asarray(inputs["be1"], f),
        W2s=np.asarray(inputs["W2s"], f), g2s=np.asarray(inputs["g2s"], f),
        be2s=np.asarray(inputs["be2s"], f),
        aw=np.asarray(inputs["aw"], f), ab=np.asarray(inputs["ab"], f),
        Wf1=np.asarray(inputs["Wf1"], f), gf1=np.asarray(inputs["gf1"], f),
        bef1=np.asarray(inputs["bef1"], f),
        Wl=np.asarray(inputs["Wl"], f), gl=np.asarray(inputs["gl"], f),
        bel=np.asarray(inputs["bel"], f),
        Wf2=np.asarray(inputs["Wf2"], f), bf2=np.asarray(inputs["bf2"], f),
    )
    maps = []
    for c in range(C):
        m = dict(base)
        m["gidx1"] = np.ascontiguousarray(cfg["gidx1"][c])
        m["gidx2"] = np.ascontiguousarray(cfg["gidx2"][c])
        m["ind"] = np.ascontiguousarray(cfg["ind"][c])
        m["indG"] = np.ascontiguousarray(cfg["indG"][c])
        maps.append(m)
    return maps


def _postprocess(cfg, results):
    C, GPC, N = cfg["C"], cfg["GPC"], cfg["N"]
    y = np.concatenate([results[c]["y_out"] for c in range(C)], axis=0)
    atom = np.zeros((N, 1), dtype=np.float32)
    for c in range(C):
        s2n = cfg["slot2node"][c]
        real = s2n >= 0
        atom[s2n[real], 0] = results[c]["atom_out"][real, 0]
    return y, atom


def run(inputs, n_graphs=1024, depth=4, trace=False):
    from concourse import bass_utils
    cfg = _preprocess(inputs, n_graphs)
    nc = _build(cfg, depth)
    maps = _in_maps(cfg, inputs, depth)
    res = bass_utils.run_bass_kernel_spmd(
        nc, maps, core_ids=list(range(N_CORES)), trace=trace,
    )
    out = _postprocess(cfg, res.results)
    return out, res


def kernel(**inputs):
    (y, atom), _ = run(inputs)
    return y, atom
